# revision 7
# baseline (speedup 1.0000x reference)
"""Trainium2 Bass kernel for per-position windowed linear (locally-connected 1D).

Computes, for x:[B,41,128], W:[41,384,128], b:[41,128]:
    out[b,s,:] = relu( concat(x[b,s-1], x[b,s], x[b,s+1]) @ W[s] + b[s] )
with zero padding at the sequence edges.

Strategy (8 NeuronCores, data-parallel over batch, 512 batches/core):
  - Host: cast x/W to bf16. Pre-transpose each x shard to [s, k, b] so the
    contraction dim (feature k) lands on SBUF partitions with contiguous DMA
    (no on-device transposes). Pre-permute W into per-group concatenated
    blocks so each matmul's moving operand is one contiguous slice.
  - Device: positions are processed in groups of 4 (one PSUM bank [128b x
    4*128f] per group x batch-subtile). For each stationary activation tile
    xT[sp] (reused across its up-to-3 consumer positions), ONE matmul with a
    wide moving operand (up to 384 cols of W chunks) writes/accumulates all
    its consumer chunks — PSUM's per-element has_written bit accumulates
    where already written and overwrites first touches. This cuts matmul
    (and LDWEIGHTS) count ~2x vs per-(position,tap) matmuls.
  - DVE ReLU from PSUM into fp16 staging, grouped contiguous DMA out,
    host upcast to fp32. (fp16 keeps 10 mantissa bits: output-rounding error
    ~4.9e-4 relative, negligible vs the bf16-matmul error ~2.3e-3, while
    halving output DMA bytes.)
  - bf16 inputs, fp32 PSUM accumulation.
"""

import os
import sys

import numpy as np
import ml_dtypes

for _p in ("/opt/trn_rl_repo", "/root/.axon_site/_ro/trn_rl_repo"):
    if os.path.isdir(_p) and _p not in sys.path:
        sys.path.append(_p)

from contextlib import ExitStack

import concourse.mybir as mybir
import concourse.tile as tile
from concourse import bacc
from concourse.bass_utils import run_bass_kernel_spmd

S = 41          # sequence positions
F = 128         # feature dim
WIN = 3         # window size
PAD = WIN // 2
N_CORES = 8
B_FULL = 4096
BPC = B_FULL // N_CORES   # 512 batches per core
NBT = BPC // 128          # 4 batch sub-tiles of 128
GS = 4                    # positions per PSUM bank group
G = (S + GS - 1) // GS    # 11 groups

_nc_cache = {}


def _group_layout():
    """Per group g: (s0, npos, [(sp, smin, ncons), ...]) where each entry is
    one matmul: stationary xT[sp], consumers s in [smin, smin+ncons) with
    window tap w = sp - s + 1."""
    out = []
    for s0 in range(0, S, GS):
        npos = min(GS, S - s0)
        ents = []
        for sp in range(max(0, s0 - 1), min(S - 1, s0 + npos) + 1):
            cons = [s for s in range(s0, s0 + npos) if abs(s - sp) <= 1]
            if cons:
                ents.append((sp, cons[0], len(cons)))
        out.append((s0, npos, ents))
    return out


_LAYOUT = _group_layout()
_WCOLS = [sum(nc_ * F for _, _, nc_ in ents) for _, _, ents in _LAYOUT]
_WTOT = sum(_WCOLS)


def _build(has_bias: bool):
    bf16 = mybir.dt.bfloat16
    f32 = mybir.dt.float32
    f16 = mybir.dt.float16
    nc = bacc.Bacc("TRN2", target_bir_lowering=False, debug=False)
    xT = nc.dram_tensor("xT", [S, F, BPC], bf16, kind="ExternalInput").ap()
    Wg = nc.dram_tensor("Wg", [F, _WTOT], bf16, kind="ExternalInput").ap()
    bias = (
        nc.dram_tensor("bias", [1, S * F], bf16, kind="ExternalInput").ap()
        if has_bias
        else None
    )
    out = nc.dram_tensor("out", [BPC, S, F], f16, kind="ExternalOutput").ap()

    with tile.TileContext(nc) as tc:
        with ExitStack() as ctx:
            xpool = ctx.enter_context(tc.tile_pool(name="xT", bufs=G))
            wpool = ctx.enter_context(tc.tile_pool(name="W", bufs=G))
            ppool = ctx.enter_context(tc.tile_pool(name="ps", bufs=8, space="PSUM"))
            opool = ctx.enter_context(tc.tile_pool(name="stage", bufs=5))

            # --- loads (issued on ACT sequencer, stores go on SP) ---
            xt, wt = [], []
            wcol0 = 0
            for g, (s0, npos, ents) in enumerate(_LAYOUT):
                tx = xpool.tile([F, GS * BPC], bf16)
                tx_v = tx[:].rearrange("k (s b) -> k s b", b=BPC)
                xT_v = xT[s0 : s0 + npos].rearrange("s k b -> k s b")
                if g == 0:
                    # split the first loads so PE can start ~2us earlier
                    nc.scalar.dma_start(tx_v[:, :1, :], xT_v[:, :1, :])
                    nc.scalar.dma_start(tx_v[:, 1:npos, :], xT_v[:, 1:, :])
                else:
                    nc.scalar.dma_start(tx_v[:, :npos, :], xT_v)
                xt.append(tx)
                tw = wpool.tile([F, max(_WCOLS)], bf16)
                if g == 0:
                    c = 2 * F  # first matmul's moving block (sp=0: 2 chunks)
                    nc.scalar.dma_start(tw[:, :c], Wg[:, wcol0 : wcol0 + c])
                    nc.scalar.dma_start(
                        tw[:, c : _WCOLS[g]], Wg[:, wcol0 + c : wcol0 + _WCOLS[g]]
                    )
                else:
                    nc.scalar.dma_start(
                        tw[:, : _WCOLS[g]], Wg[:, wcol0 : wcol0 + _WCOLS[g]]
                    )
                wt.append(tw)
                wcol0 += _WCOLS[g]

            if has_bias:
                bpool = ctx.enter_context(tc.tile_pool(name="bias", bufs=1))
                bias_sb = bpool.tile([1, S * F], bf16)
                nc.scalar.dma_start(bias_sb[:], bias[:])
                ones = bpool.tile([1, F], bf16)
                nc.vector.memset(ones[:], 1.0)

            # iterated (p, t, s, f) to match the SBUF-side flatten order
            out_r = out.rearrange("(t p) s f -> p t s f", p=128)

            # --- compute ---
            for g, (s0, npos, ents) in enumerate(_LAYOUT):
                stage = opool.tile([128, NBT * GS * F], f16)
                for bt in range(NBT):
                    ps = ppool.tile([128, GS * F], f32)
                    n_mm = len(ents) + (1 if has_bias else 0)
                    wcol = 0
                    for j, (sp, smin, ncons) in enumerate(ents):
                        gi, sub = divmod(sp, GS)
                        lhsT = xt[gi][:, sub * BPC + bt * 128 : sub * BPC + (bt + 1) * 128]
                        c0 = (smin - s0) * F
                        nc.tensor.matmul(
                            ps[:, c0 : c0 + ncons * F],
                            lhsT=lhsT,
                            rhs=wt[g][:, wcol : wcol + ncons * F],
                            start=(j == 0),
                            stop=(j == n_mm - 1),
                        )
                        wcol += ncons * F
                    if has_bias:
                        nc.tensor.matmul(
                            ps[:, : npos * F],
                            lhsT=ones[:],
                            rhs=bias_sb[:, s0 * F : (s0 + npos) * F],
                            start=False,
                            stop=True,
                        )
                    nc.vector.tensor_scalar_max(
                        stage[:, bt * GS * F : bt * GS * F + npos * F],
                        ps[:, : npos * F],
                        0.0,
                    )
                # one store per group: SBUF [p, (bt, s_local, f)] -> DRAM
                stage_v = stage[:].rearrange("p (t s f) -> p t s f", t=NBT, f=F)
                nc.sync.dma_start(
                    out_r[:, :, s0 : s0 + npos, :],
                    stage_v[:, :, :npos, :],
                )

    nc.compile()
    return nc


def _get_nc(has_bias: bool):
    if has_bias not in _nc_cache:
        _nc_cache[has_bias] = _build(has_bias)
    return _nc_cache[has_bias]


def _prep_in_maps(inputs: np.ndarray, W: np.ndarray, b: np.ndarray):
    has_bias = bool(np.any(b))
    Wb = W.astype(ml_dtypes.bfloat16)
    blocks = []
    for s0, npos, ents in _LAYOUT:
        for sp, smin, ncons in ents:
            for s in range(smin, smin + ncons):
                w = sp - s + 1
                blocks.append(Wb[s, w * F : (w + 1) * F, :])  # [128k, 128f]
    Wg = np.ascontiguousarray(np.concatenate(blocks, axis=1))  # [128, _WTOT]
    assert Wg.shape == (F, _WTOT), Wg.shape
    xb = inputs.astype(ml_dtypes.bfloat16)
    bias = (
        np.ascontiguousarray(b.astype(ml_dtypes.bfloat16).reshape(1, S * F))
        if has_bias
        else None
    )
    in_maps = []
    for c in range(N_CORES):
        shard = xb[c * BPC : (c + 1) * BPC]
        m = {"xT": np.ascontiguousarray(shard.transpose(1, 2, 0)), "Wg": Wg}
        if has_bias:
            m["bias"] = bias
        in_maps.append(m)
    return in_maps, has_bias


def kernel(inputs: np.ndarray, W: np.ndarray, b: np.ndarray) -> np.ndarray:
    inputs = np.asarray(inputs)
    W = np.asarray(W)
    b = np.asarray(b)
    assert inputs.shape == (B_FULL, S, F), inputs.shape
    in_maps, has_bias = _prep_in_maps(inputs, W, b)
    nc = _get_nc(has_bias)
    res = run_bass_kernel_spmd(nc, in_maps, list(range(N_CORES)))
    out = np.concatenate([r["out"] for r in res.results], axis=0)
    return np.ascontiguousarray(out.astype(np.float32))


# revision 8
# speedup vs baseline: 1.0544x; 1.0544x over previous
"""Trainium2 Bass kernel for per-position windowed linear (locally-connected 1D).

Computes, for x:[B,41,128], W:[41,384,128], b:[41,128]:
    out[b,s,:] = relu( concat(x[b,s-1], x[b,s], x[b,s+1]) @ W[s] + b[s] )
with zero padding at the sequence edges.

Strategy (8 NeuronCores, data-parallel over batch, 512 batches/core):
  - Host: cast x/W to bf16. Pre-transpose each x shard to [s, k, b] so the
    contraction dim (feature k) lands on SBUF partitions with contiguous DMA
    (no on-device transposes). Pre-permute W into per-group concatenated
    blocks so each matmul's moving operand is one contiguous slice.
  - Device: positions are processed in groups of 4 (one PSUM bank [128b x
    4*128f] per group x batch-subtile). For each stationary activation tile
    xT[sp] (reused across its up-to-3 consumer positions), ONE matmul with a
    wide moving operand (up to 384 cols of W chunks) writes/accumulates all
    its consumer chunks — PSUM's per-element has_written bit accumulates
    where already written and overwrites first touches. This cuts matmul
    (and LDWEIGHTS) count ~2x vs per-(position,tap) matmuls.
  - DVE ReLU from PSUM into fp16 staging, grouped contiguous DMA out,
    host upcast to fp32. (fp16 keeps 10 mantissa bits: output-rounding error
    ~4.9e-4 relative, negligible vs the bf16-matmul error ~2.3e-3, while
    halving output DMA bytes.)
  - bf16 inputs, fp32 PSUM accumulation.
"""

import os
import sys

import numpy as np
import ml_dtypes

for _p in ("/opt/trn_rl_repo", "/root/.axon_site/_ro/trn_rl_repo"):
    if os.path.isdir(_p) and _p not in sys.path:
        sys.path.append(_p)

from contextlib import ExitStack

import concourse.mybir as mybir
import concourse.tile as tile
from concourse import bacc
from concourse.bass_utils import run_bass_kernel_spmd

S = 41          # sequence positions
F = 128         # feature dim
WIN = 3         # window size
PAD = WIN // 2
N_CORES = 8
B_FULL = 4096
BPC = B_FULL // N_CORES   # 512 batches per core
NBT = BPC // 128          # 4 batch sub-tiles of 128
GS = 4                    # positions per PSUM bank group
G = (S + GS - 1) // GS    # 11 groups

_nc_cache = {}


def _group_layout():
    """Per group g: (s0, npos, [(sp, smin, ncons), ...]) where each entry is
    one matmul: stationary xT[sp], consumers s in [smin, smin+ncons) with
    window tap w = sp - s + 1."""
    out = []
    for s0 in range(0, S, GS):
        npos = min(GS, S - s0)
        ents = []
        for sp in range(max(0, s0 - 1), min(S - 1, s0 + npos) + 1):
            cons = [s for s in range(s0, s0 + npos) if abs(s - sp) <= 1]
            if cons:
                ents.append((sp, cons[0], len(cons)))
        out.append((s0, npos, ents))
    return out


_LAYOUT = _group_layout()
_WCOLS = [sum(nc_ * F for _, _, nc_ in ents) for _, _, ents in _LAYOUT]
_WTOT = sum(_WCOLS)


def _build(has_bias: bool):
    bf16 = mybir.dt.bfloat16
    f32 = mybir.dt.float32
    f16 = mybir.dt.float16
    nc = bacc.Bacc("TRN2", target_bir_lowering=False, debug=False)
    xT = nc.dram_tensor("xT", [S, F, BPC], bf16, kind="ExternalInput").ap()
    Wg = nc.dram_tensor("Wg", [F, _WTOT], bf16, kind="ExternalInput").ap()
    bias = (
        nc.dram_tensor("bias", [1, S * F], bf16, kind="ExternalInput").ap()
        if has_bias
        else None
    )
    out = nc.dram_tensor("out", [BPC, S, F], f16, kind="ExternalOutput").ap()

    with tile.TileContext(nc) as tc:
        with ExitStack() as ctx:
            xpool = ctx.enter_context(tc.tile_pool(name="xT", bufs=G))
            wpool = ctx.enter_context(tc.tile_pool(name="W", bufs=G))
            ppool = ctx.enter_context(tc.tile_pool(name="ps", bufs=8, space="PSUM"))
            opool = ctx.enter_context(tc.tile_pool(name="stage", bufs=5))

            # --- loads (issued on ACT sequencer, stores go on SP) ---
            xt, wt = [], []
            wcol0 = 0
            for g, (s0, npos, ents) in enumerate(_LAYOUT):
                tx = xpool.tile([F, GS * BPC], bf16)
                nc.scalar.dma_start(
                    tx[:].rearrange("k (s b) -> k s b", b=BPC)[:, :npos, :],
                    xT[s0 : s0 + npos].rearrange("s k b -> k s b"),
                )
                xt.append(tx)
                tw = wpool.tile([F, max(_WCOLS)], bf16)
                nc.scalar.dma_start(tw[:, : _WCOLS[g]], Wg[:, wcol0 : wcol0 + _WCOLS[g]])
                wt.append(tw)
                wcol0 += _WCOLS[g]

            if has_bias:
                bpool = ctx.enter_context(tc.tile_pool(name="bias", bufs=1))
                bias_sb = bpool.tile([1, S * F], bf16)
                nc.scalar.dma_start(bias_sb[:], bias[:])
                ones = bpool.tile([1, F], bf16)
                nc.vector.memset(ones[:], 1.0)

            # iterated (p, t, s, f) to match the SBUF-side flatten order
            out_r = out.rearrange("(t p) s f -> p t s f", p=128)

            # --- compute ---
            for g, (s0, npos, ents) in enumerate(_LAYOUT):
                stage = opool.tile([128, NBT * GS * F], f16)
                for bt in range(NBT):
                    ps = ppool.tile([128, GS * F], f32)
                    n_mm = len(ents) + (1 if has_bias else 0)
                    wcol = 0
                    for j, (sp, smin, ncons) in enumerate(ents):
                        gi, sub = divmod(sp, GS)
                        lhsT = xt[gi][:, sub * BPC + bt * 128 : sub * BPC + (bt + 1) * 128]
                        c0 = (smin - s0) * F
                        nc.tensor.matmul(
                            ps[:, c0 : c0 + ncons * F],
                            lhsT=lhsT,
                            rhs=wt[g][:, wcol : wcol + ncons * F],
                            start=(j == 0),
                            stop=(j == n_mm - 1),
                        )
                        wcol += ncons * F
                    if has_bias:
                        nc.tensor.matmul(
                            ps[:, : npos * F],
                            lhsT=ones[:],
                            rhs=bias_sb[:, s0 * F : (s0 + npos) * F],
                            start=False,
                            stop=True,
                        )
                    nc.vector.tensor_scalar_max(
                        stage[:, bt * GS * F : bt * GS * F + npos * F],
                        ps[:, : npos * F],
                        0.0,
                    )
                # one store per group: SBUF [p, (bt, s_local, f)] -> DRAM
                stage_v = stage[:].rearrange("p (t s f) -> p t s f", t=NBT, f=F)
                nc.sync.dma_start(
                    out_r[:, :, s0 : s0 + npos, :],
                    stage_v[:, :, :npos, :],
                )

    nc.compile()
    return nc


def _get_nc(has_bias: bool):
    if has_bias not in _nc_cache:
        _nc_cache[has_bias] = _build(has_bias)
    return _nc_cache[has_bias]


def _prep_in_maps(inputs: np.ndarray, W: np.ndarray, b: np.ndarray):
    has_bias = bool(np.any(b))
    Wb = W.astype(ml_dtypes.bfloat16)
    blocks = []
    for s0, npos, ents in _LAYOUT:
        for sp, smin, ncons in ents:
            for s in range(smin, smin + ncons):
                w = sp - s + 1
                blocks.append(Wb[s, w * F : (w + 1) * F, :])  # [128k, 128f]
    Wg = np.ascontiguousarray(np.concatenate(blocks, axis=1))  # [128, _WTOT]
    assert Wg.shape == (F, _WTOT), Wg.shape
    xb = inputs.astype(ml_dtypes.bfloat16)
    bias = (
        np.ascontiguousarray(b.astype(ml_dtypes.bfloat16).reshape(1, S * F))
        if has_bias
        else None
    )
    in_maps = []
    for c in range(N_CORES):
        shard = xb[c * BPC : (c + 1) * BPC]
        m = {"xT": np.ascontiguousarray(shard.transpose(1, 2, 0)), "Wg": Wg}
        if has_bias:
            m["bias"] = bias
        in_maps.append(m)
    return in_maps, has_bias


def kernel(inputs: np.ndarray, W: np.ndarray, b: np.ndarray) -> np.ndarray:
    inputs = np.asarray(inputs)
    W = np.asarray(W)
    b = np.asarray(b)
    assert inputs.shape == (B_FULL, S, F), inputs.shape
    in_maps, has_bias = _prep_in_maps(inputs, W, b)
    nc = _get_nc(has_bias)
    res = run_bass_kernel_spmd(nc, in_maps, list(range(N_CORES)))
    out = np.concatenate([r["out"] for r in res.results], axis=0)
    return np.ascontiguousarray(out.astype(np.float32))


# revision 11
# speedup vs baseline: 1.0586x; 1.0039x over previous
"""Trainium2 Bass kernel for per-position windowed linear (locally-connected 1D).

Computes, for x:[B,41,128], W:[41,384,128], b:[41,128]:
    out[b,s,:] = relu( concat(x[b,s-1], x[b,s], x[b,s+1]) @ W[s] + b[s] )
with zero padding at the sequence edges.

Strategy (8 NeuronCores, data-parallel over batch, 512 batches/core):
  - Host: cast x/W to bf16. Pre-transpose each x shard to [s, k, b] so the
    contraction dim (feature k) lands on SBUF partitions with contiguous DMA
    (no on-device transposes). Pre-permute W into per-group concatenated
    blocks so each matmul's moving operand is one contiguous slice.
  - Device: positions are processed in groups of 4 (one PSUM bank [128b x
    4*128f] per group x batch-subtile). For each stationary activation tile
    xT[sp] (reused across its up-to-3 consumer positions), ONE matmul with a
    wide moving operand (up to 384 cols of W chunks) writes/accumulates all
    its consumer chunks — PSUM's per-element has_written bit accumulates
    where already written and overwrites first touches. This cuts matmul
    (and LDWEIGHTS) count ~2x vs per-(position,tap) matmuls.
  - DVE ReLU from PSUM into fp16 staging, grouped contiguous DMA out,
    host upcast to fp32. (fp16 keeps 10 mantissa bits: output-rounding error
    ~4.9e-4 relative, negligible vs the bf16-matmul error ~2.3e-3, while
    halving output DMA bytes.)
  - bf16 inputs, fp32 PSUM accumulation.
"""

import os
import sys

import numpy as np
import ml_dtypes

for _p in ("/opt/trn_rl_repo", "/root/.axon_site/_ro/trn_rl_repo"):
    if os.path.isdir(_p) and _p not in sys.path:
        sys.path.append(_p)

from contextlib import ExitStack

import concourse.mybir as mybir
import concourse.tile as tile
from concourse import bacc
from concourse.bass_utils import run_bass_kernel_spmd

S = 41          # sequence positions
F = 128         # feature dim
WIN = 3         # window size
PAD = WIN // 2
N_CORES = 8
B_FULL = 4096
BPC = B_FULL // N_CORES   # 512 batches per core
NBT = BPC // 128          # 4 batch sub-tiles of 128
GS = 4                    # positions per PSUM bank group
G = (S + GS - 1) // GS    # 11 groups

_nc_cache = {}


def _group_layout():
    """Per group g: (s0, npos, [(sp, smin, ncons), ...]) where each entry is
    one matmul: stationary xT[sp], consumers s in [smin, smin+ncons) with
    window tap w = sp - s + 1."""
    out = []
    for s0 in range(0, S, GS):
        npos = min(GS, S - s0)
        ents = []
        for sp in range(max(0, s0 - 1), min(S - 1, s0 + npos) + 1):
            cons = [s for s in range(s0, s0 + npos) if abs(s - sp) <= 1]
            if cons:
                ents.append((sp, cons[0], len(cons)))
        out.append((s0, npos, ents))
    return out


_LAYOUT = _group_layout()
_WCOLS = [sum(nc_ * F for _, _, nc_ in ents) for _, _, ents in _LAYOUT]
_WTOT = sum(_WCOLS)


def _build(has_bias: bool):
    bf16 = mybir.dt.bfloat16
    f32 = mybir.dt.float32
    f16 = mybir.dt.float16
    nc = bacc.Bacc("TRN2", target_bir_lowering=False, debug=False)
    xT = nc.dram_tensor("xT", [S, F, BPC], bf16, kind="ExternalInput").ap()
    Wg = nc.dram_tensor("Wg", [F, _WTOT], bf16, kind="ExternalInput").ap()
    bias = (
        nc.dram_tensor("bias", [1, S * F], bf16, kind="ExternalInput").ap()
        if has_bias
        else None
    )
    out = nc.dram_tensor("out", [BPC, S, F], f16, kind="ExternalOutput").ap()

    with tile.TileContext(nc) as tc:
        with ExitStack() as ctx:
            xpool = ctx.enter_context(tc.tile_pool(name="xT", bufs=G))
            wpool = ctx.enter_context(tc.tile_pool(name="W", bufs=G))
            ppool = ctx.enter_context(tc.tile_pool(name="ps", bufs=8, space="PSUM"))
            opool = ctx.enter_context(tc.tile_pool(name="stage", bufs=5))

            # --- loads (issued on ACT sequencer, stores go on SP) ---
            xt, wt = [], []
            wcol0 = 0
            for g, (s0, npos, ents) in enumerate(_LAYOUT):
                tx = xpool.tile([F, GS * BPC], bf16)
                nc.scalar.dma_start(
                    tx[:].rearrange("k (s b) -> k s b", b=BPC)[:, :npos, :],
                    xT[s0 : s0 + npos].rearrange("s k b -> k s b"),
                )
                xt.append(tx)
                tw = wpool.tile([F, max(_WCOLS)], bf16)
                nc.scalar.dma_start(tw[:, : _WCOLS[g]], Wg[:, wcol0 : wcol0 + _WCOLS[g]])
                wt.append(tw)
                wcol0 += _WCOLS[g]

            if has_bias:
                bpool = ctx.enter_context(tc.tile_pool(name="bias", bufs=1))
                bias_sb = bpool.tile([1, S * F], bf16)
                nc.scalar.dma_start(bias_sb[:], bias[:])
                ones = bpool.tile([1, F], bf16)
                nc.vector.memset(ones[:], 1.0)

            # iterated (p, t, s, f) to match the SBUF-side flatten order
            out_r = out.rearrange("(t p) s f -> p t s f", p=128)

            # --- compute ---
            for g, (s0, npos, ents) in enumerate(_LAYOUT):
                stage = opool.tile([128, NBT * GS * F], f16)
                for bt in range(NBT):
                    ps = ppool.tile([128, GS * F], f32)
                    n_mm = len(ents) + (1 if has_bias else 0)
                    wcol = 0
                    for j, (sp, smin, ncons) in enumerate(ents):
                        gi, sub = divmod(sp, GS)
                        lhsT = xt[gi][:, sub * BPC + bt * 128 : sub * BPC + (bt + 1) * 128]
                        c0 = (smin - s0) * F
                        nc.tensor.matmul(
                            ps[:, c0 : c0 + ncons * F],
                            lhsT=lhsT,
                            rhs=wt[g][:, wcol : wcol + ncons * F],
                            start=(j == 0),
                            stop=(j == n_mm - 1),
                        )
                        wcol += ncons * F
                    if has_bias:
                        nc.tensor.matmul(
                            ps[:, : npos * F],
                            lhsT=ones[:],
                            rhs=bias_sb[:, s0 * F : (s0 + npos) * F],
                            start=False,
                            stop=True,
                        )
                    nc.vector.tensor_scalar_max(
                        stage[:, bt * GS * F : bt * GS * F + npos * F],
                        ps[:, : npos * F],
                        0.0,
                    )
                # stores per group, split in two so the first half can start
                # as soon as its two batch-subtiles' relus are done
                stage_v = stage[:].rearrange("p (t s f) -> p t s f", t=NBT, f=F)
                for o in range(2):
                    h = NBT // 2
                    nc.sync.dma_start(
                        out_r[:, o * h : (o + 1) * h, s0 : s0 + npos, :],
                        stage_v[:, o * h : (o + 1) * h, :npos, :],
                    )

    nc.compile()
    return nc


def _get_nc(has_bias: bool):
    if has_bias not in _nc_cache:
        _nc_cache[has_bias] = _build(has_bias)
    return _nc_cache[has_bias]


def _prep_in_maps(inputs: np.ndarray, W: np.ndarray, b: np.ndarray):
    has_bias = bool(np.any(b))
    Wb = W.astype(ml_dtypes.bfloat16)
    blocks = []
    for s0, npos, ents in _LAYOUT:
        for sp, smin, ncons in ents:
            for s in range(smin, smin + ncons):
                w = sp - s + 1
                blocks.append(Wb[s, w * F : (w + 1) * F, :])  # [128k, 128f]
    Wg = np.ascontiguousarray(np.concatenate(blocks, axis=1))  # [128, _WTOT]
    assert Wg.shape == (F, _WTOT), Wg.shape
    xb = inputs.astype(ml_dtypes.bfloat16)
    bias = (
        np.ascontiguousarray(b.astype(ml_dtypes.bfloat16).reshape(1, S * F))
        if has_bias
        else None
    )
    in_maps = []
    for c in range(N_CORES):
        shard = xb[c * BPC : (c + 1) * BPC]
        m = {"xT": np.ascontiguousarray(shard.transpose(1, 2, 0)), "Wg": Wg}
        if has_bias:
            m["bias"] = bias
        in_maps.append(m)
    return in_maps, has_bias


def kernel(inputs: np.ndarray, W: np.ndarray, b: np.ndarray) -> np.ndarray:
    inputs = np.asarray(inputs)
    W = np.asarray(W)
    b = np.asarray(b)
    assert inputs.shape == (B_FULL, S, F), inputs.shape
    in_maps, has_bias = _prep_in_maps(inputs, W, b)
    nc = _get_nc(has_bias)
    res = run_bass_kernel_spmd(nc, in_maps, list(range(N_CORES)))
    out = np.concatenate([r["out"] for r in res.results], axis=0)
    return np.ascontiguousarray(out.astype(np.float32))


# revision 12
# speedup vs baseline: 1.0623x; 1.0035x over previous
"""Trainium2 Bass kernel for per-position windowed linear (locally-connected 1D).

Computes, for x:[B,41,128], W:[41,384,128], b:[41,128]:
    out[b,s,:] = relu( concat(x[b,s-1], x[b,s], x[b,s+1]) @ W[s] + b[s] )
with zero padding at the sequence edges.

Strategy (8 NeuronCores, data-parallel over batch, 512 batches/core):
  - Host: cast x/W to bf16. Pre-transpose each x shard to [s, k, b] so the
    contraction dim (feature k) lands on SBUF partitions with contiguous DMA
    (no on-device transposes). Pre-permute W into per-group concatenated
    blocks so each matmul's moving operand is one contiguous slice.
  - Device: positions are processed in groups of 4 (one PSUM bank [128b x
    4*128f] per group x batch-subtile). For each stationary activation tile
    xT[sp] (reused across its up-to-3 consumer positions), ONE matmul with a
    wide moving operand (up to 384 cols of W chunks) writes/accumulates all
    its consumer chunks — PSUM's per-element has_written bit accumulates
    where already written and overwrites first touches. This cuts matmul
    (and LDWEIGHTS) count ~2x vs per-(position,tap) matmuls.
  - DVE ReLU from PSUM into fp16 staging, grouped contiguous DMA out,
    host upcast to fp32. (fp16 keeps 10 mantissa bits: output-rounding error
    ~4.9e-4 relative, negligible vs the bf16-matmul error ~2.3e-3, while
    halving output DMA bytes.)
  - bf16 inputs, fp32 PSUM accumulation.
"""

import os
import sys

import numpy as np
import ml_dtypes

for _p in ("/opt/trn_rl_repo", "/root/.axon_site/_ro/trn_rl_repo"):
    if os.path.isdir(_p) and _p not in sys.path:
        sys.path.append(_p)

from contextlib import ExitStack

import concourse.mybir as mybir
import concourse.tile as tile
from concourse import bacc
from concourse.bass_utils import run_bass_kernel_spmd

S = 41          # sequence positions
F = 128         # feature dim
WIN = 3         # window size
PAD = WIN // 2
N_CORES = 8
B_FULL = 4096
BPC = B_FULL // N_CORES   # 512 batches per core
NBT = BPC // 128          # 4 batch sub-tiles of 128
GS = 4                    # positions per PSUM bank group
G = (S + GS - 1) // GS    # 11 groups

_nc_cache = {}


def _group_layout():
    """Per group g: (s0, npos, [(sp, smin, ncons), ...]) where each entry is
    one matmul: stationary xT[sp], consumers s in [smin, smin+ncons) with
    window tap w = sp - s + 1."""
    out = []
    for s0 in range(0, S, GS):
        npos = min(GS, S - s0)
        ents = []
        for sp in range(max(0, s0 - 1), min(S - 1, s0 + npos) + 1):
            cons = [s for s in range(s0, s0 + npos) if abs(s - sp) <= 1]
            if cons:
                ents.append((sp, cons[0], len(cons)))
        out.append((s0, npos, ents))
    return out


_LAYOUT = _group_layout()
_WCOLS = [sum(nc_ * F for _, _, nc_ in ents) for _, _, ents in _LAYOUT]
_WTOT = sum(_WCOLS)


def _build(has_bias: bool):
    bf16 = mybir.dt.bfloat16
    f32 = mybir.dt.float32
    f16 = mybir.dt.float16
    nc = bacc.Bacc("TRN2", target_bir_lowering=False, debug=False)
    xT = nc.dram_tensor("xT", [S, F, BPC], bf16, kind="ExternalInput").ap()
    Wg = nc.dram_tensor("Wg", [F, _WTOT], bf16, kind="ExternalInput").ap()
    bias = (
        nc.dram_tensor("bias", [1, S * F], bf16, kind="ExternalInput").ap()
        if has_bias
        else None
    )
    out = nc.dram_tensor("out", [BPC, S, F], f16, kind="ExternalOutput").ap()

    with tile.TileContext(nc) as tc:
        with ExitStack() as ctx:
            xpool = ctx.enter_context(tc.tile_pool(name="xT", bufs=G))
            wpool = ctx.enter_context(tc.tile_pool(name="W", bufs=G))
            ppool = ctx.enter_context(tc.tile_pool(name="ps", bufs=8, space="PSUM"))
            opool = ctx.enter_context(tc.tile_pool(name="stage", bufs=5))

            # --- loads (issued on ACT sequencer, stores go on SP) ---
            xt, wt = [], []
            wcol0 = 0
            for g, (s0, npos, ents) in enumerate(_LAYOUT):
                tx = xpool.tile([F, GS * BPC], bf16)
                # first load on SP: shorter issue chain, trims the head ~150ns
                eng = nc.sync if g == 0 else nc.scalar
                eng.dma_start(
                    tx[:].rearrange("k (s b) -> k s b", b=BPC)[:, :npos, :],
                    xT[s0 : s0 + npos].rearrange("s k b -> k s b"),
                )
                xt.append(tx)
                tw = wpool.tile([F, max(_WCOLS)], bf16)
                nc.scalar.dma_start(tw[:, : _WCOLS[g]], Wg[:, wcol0 : wcol0 + _WCOLS[g]])
                wt.append(tw)
                wcol0 += _WCOLS[g]

            if has_bias:
                bpool = ctx.enter_context(tc.tile_pool(name="bias", bufs=1))
                bias_sb = bpool.tile([1, S * F], bf16)
                nc.scalar.dma_start(bias_sb[:], bias[:])
                ones = bpool.tile([1, F], bf16)
                nc.vector.memset(ones[:], 1.0)

            # iterated (p, t, s, f) to match the SBUF-side flatten order
            out_r = out.rearrange("(t p) s f -> p t s f", p=128)

            # --- compute ---
            for g, (s0, npos, ents) in enumerate(_LAYOUT):
                stage = opool.tile([128, NBT * GS * F], f16)
                for bt in range(NBT):
                    ps = ppool.tile([128, GS * F], f32)
                    n_mm = len(ents) + (1 if has_bias else 0)
                    wcol = 0
                    for j, (sp, smin, ncons) in enumerate(ents):
                        gi, sub = divmod(sp, GS)
                        lhsT = xt[gi][:, sub * BPC + bt * 128 : sub * BPC + (bt + 1) * 128]
                        c0 = (smin - s0) * F
                        nc.tensor.matmul(
                            ps[:, c0 : c0 + ncons * F],
                            lhsT=lhsT,
                            rhs=wt[g][:, wcol : wcol + ncons * F],
                            start=(j == 0),
                            stop=(j == n_mm - 1),
                        )
                        wcol += ncons * F
                    if has_bias:
                        nc.tensor.matmul(
                            ps[:, : npos * F],
                            lhsT=ones[:],
                            rhs=bias_sb[:, s0 * F : (s0 + npos) * F],
                            start=False,
                            stop=True,
                        )
                    nc.vector.tensor_scalar_max(
                        stage[:, bt * GS * F : bt * GS * F + npos * F],
                        ps[:, : npos * F],
                        0.0,
                    )
                # stores per group, split in two so the first half can start
                # as soon as its two batch-subtiles' relus are done
                stage_v = stage[:].rearrange("p (t s f) -> p t s f", t=NBT, f=F)
                for o in range(2):
                    h = NBT // 2
                    nc.sync.dma_start(
                        out_r[:, o * h : (o + 1) * h, s0 : s0 + npos, :],
                        stage_v[:, o * h : (o + 1) * h, :npos, :],
                    )

    nc.compile()
    return nc


def _get_nc(has_bias: bool):
    if has_bias not in _nc_cache:
        _nc_cache[has_bias] = _build(has_bias)
    return _nc_cache[has_bias]


def _prep_in_maps(inputs: np.ndarray, W: np.ndarray, b: np.ndarray):
    has_bias = bool(np.any(b))
    Wb = W.astype(ml_dtypes.bfloat16)
    blocks = []
    for s0, npos, ents in _LAYOUT:
        for sp, smin, ncons in ents:
            for s in range(smin, smin + ncons):
                w = sp - s + 1
                blocks.append(Wb[s, w * F : (w + 1) * F, :])  # [128k, 128f]
    Wg = np.ascontiguousarray(np.concatenate(blocks, axis=1))  # [128, _WTOT]
    assert Wg.shape == (F, _WTOT), Wg.shape
    xb = inputs.astype(ml_dtypes.bfloat16)
    bias = (
        np.ascontiguousarray(b.astype(ml_dtypes.bfloat16).reshape(1, S * F))
        if has_bias
        else None
    )
    in_maps = []
    for c in range(N_CORES):
        shard = xb[c * BPC : (c + 1) * BPC]
        m = {"xT": np.ascontiguousarray(shard.transpose(1, 2, 0)), "Wg": Wg}
        if has_bias:
            m["bias"] = bias
        in_maps.append(m)
    return in_maps, has_bias


def kernel(inputs: np.ndarray, W: np.ndarray, b: np.ndarray) -> np.ndarray:
    inputs = np.asarray(inputs)
    W = np.asarray(W)
    b = np.asarray(b)
    assert inputs.shape == (B_FULL, S, F), inputs.shape
    in_maps, has_bias = _prep_in_maps(inputs, W, b)
    nc = _get_nc(has_bias)
    res = run_bass_kernel_spmd(nc, in_maps, list(range(N_CORES)))
    out = np.concatenate([r["out"] for r in res.results], axis=0)
    return np.ascontiguousarray(out.astype(np.float32))
